# revision 29
# baseline (speedup 1.0000x reference)
"""CodeWiseAttention kernel for Trainium2 (8 NeuronCores, label-dim sharded).

m[b,n,:] = softmax(label_feature[n] @ x[b].T) @ x[b]

Sharding: label rows N=8922 split across 8 cores (1116/core); x replicated.
Per core, per batch b:
  mm1 (fp16):  S^T[l,n] = xT[e,l].T @ labT[e,n]    (xT, labT pre-transposed
               on host; fp16 runs the PE at 1 cycle/row vs ~3.5 for fp32)
  exp:  split across BOTH ScalarE and VectorE to break the ACT-engine
        ridge (ACT alone would need 1116 cols x 0.83ns = 930ns/chunk plus
        ~350ns fixed -- over the ~980ns PE chunk time):
        - ScalarE: exp(s-30) -> bf16 for n[0:712] and the n[1024:1116]
          tail (pack-batched, 5 l-chunks per ACTIVATE).
        - VectorE: n[712:1024] via an exact-exponent bit-trick.  labT for
          these columns is pre-scaled by log2(e) on host so PSUM holds
          s' = s*log2e.  Pass 1 (tensor_single_scalar): E = int16(s'+B-.5)
          (HW converter rounds-to-nearest; the -0.5 pre-bias makes it a
          floor).  Pass 2 (custom 7-stage DVE op ANT_EXPBITS_QC):
          bits = (E + q(g))*128 with g = s'+B - E in [0,1) and
          q(g) = g*(1 + a*(g-1)) ~ 2^g - 1; the int16 bits ARE
          bf16(2^(s'+B-127)).  Per-column scale e^shift cancels in the
          host-side division by Z.  Max |exp err| ~0.6%; end-to-end rel
          err vs f64 softmax measured 6.7e-3 offline (= the bf16-output
          floor; plain Schraudolph at 3% ripple measured 1.4e-2).
  mm2 (bf16):  Uaug^T[e',n] += xa[l,e'].T @ expS^T[l,n] accumulated over l,
               where xa has a ones column so row 100 of Uaug = Z.
  out: VectorE copies Uaug^T -> SBUF as bf16 (paired 2 batches per tile);
       DMA to DRAM m[EA, B, NSP] so each partition's pair is one contiguous
       4464B packet.  All SBUF->DRAM descriptors of one HW DGE queue
       execute on a single DMA engine (~196ns/packet), so pair DMAs are
       split by partitions across the qSP (nc.sync) and qAct (nc.scalar)
       queues; the final batch is 3-way split (sync/scalar/gpsimd-swdge)
       to shorten the end-of-run drain.  Host divides by Z and transposes.

Pipeline: the exp engines run at ~1.0us/chunk, i.e. AT the PE chunk time,
so mm2 for chunk c is emitted TWO chunks late (PE slot c+1 = [mm1(c+2),
mm2(c)]) giving exp(c) a ~2-slot window; semaphore/queue latency then
never stalls the PE.  mm2 order within a batch is just accumulation
order, so the extra delay is free.  The U copy (bank WAR for the next
batch) is split in halves interleaved between the next chunk's two
VectorE exp instructions.

PSUM layout (8 banks x 512 fp32 per partition):
  b0 [0:512]      scores buf A, n[0:512]     } ACT reads [0:712),
  b1 [512:1024]   scores buf A, n[512:1024]  } DVE reads [712:1024)
  b2 [1024:1536]  scores buf B, n[0:512]
  b3 [1536:2048]  scores buf B, n[512:1024]
  b4 [2048:2560]  U accum, n[0:512]
  b5 [2560:3072]  U accum, n[512:1024]
  b6 [3072:3532]  "pack" scores: n[1024:1116] for 5 l-chunks at once
  b7 [3584:3676]  U accum, n[1024:1116]
No bank is PE-written while ScalarE/VectorE reads it (fatal collision);
matmul outputs each sit inside one bank; accumulator banks never see
start=True from score writes.
"""
import numpy as np
import ml_dtypes
from contextlib import ExitStack

import concourse.tile as tile
from concourse import bacc, mybir
from concourse.bass_utils import run_bass_kernel_spmd

F32 = mybir.dt.float32
F16 = mybir.dt.float16
BF16 = mybir.dt.bfloat16
I16 = mybir.dt.int16

B, L, E = 8, 2500, 100
LP = 2520          # L padded; pad rows have xa=0 (incl. ones col) so they
                   # contribute nothing to U or Z even though exp != 0
N_TOTAL = 8922
NCORES = 8
NS = 1116          # label rows per core (core 7: 1110 real)
NSP = 1116
NMAIN = 1024       # n columns in the main (512,512) loop
NT = 92            # tail n columns handled by packs
W = 312            # n columns handled by the VectorE exp bit-trick
NACT = NMAIN - W   # 712: n columns on ScalarE in the main loop
PSPLIT = 276       # pack-act split point (3 pieces / 2 pieces)
NWARM = 18         # PE warm-up matmuls (ramp the DVFS during input DMA)
LC = 126           # l-chunk rows (partition dim of S^T)
NLC = LP // LC     # 20 l-chunks
PACK = 5           # l-chunks per tail pack
NPACK = NLC // PACK
EA = E + 1         # x augmented with ones column
EXP_BIAS = -30.0

LOG2E = 1.4426950408889634
DVE_SHIFT = 28.0                      # e^{s-shift} scale; cancels in Z-div
B1 = 127.0 - DVE_SHIFT * LOG2E        # 86.6045...
B1E = B1 - 0.5                        # pass-1 scalar: rne(u-0.5) == floor(u)
B1C = B1                              # pass-2 C1
A128 = 28.0                           # 128*a of q(g)=g(1+a(g-1))

SA0, SB0 = 0, 1024     # main score buffer offsets (f32 elems)
U0 = 2048              # main U accumulator offset
PS0 = 3072             # pack score offset
UT0 = 3584             # tail U accumulator offset

TRACE = False
LAST_RESULT = None

_NC = []

_EXP_NAME = "ANT_EXPBITS_QC"


def _register_expbits():
    """Register the custom DVE op (documented extension point: append to
    dve_ops.OPS).  bits = g*(C2*(g-1) + C0) + Src1*C0 with g = Src0+C1-Src1;
    7 ALU stages.  Idempotent across calls/processes."""
    import concourse.dve_ops as dvo
    from concourse.dve_spec import Spec, Src0, Src1, C0, C1, C2, One, lower
    from concourse.dve_uop import DveOpSpec

    for op in dvo.OPS:
        if op.name == _EXP_NAME:
            return op

    u = Src0 + C1
    g = u - Src1
    m3 = (g - One) * C2 + C0
    body = g * m3 + Src1 * C0

    def ref(in0, in1, s0, s1, imm2):
        gg = (np.asarray(in0, np.float32) + np.float32(s1)) - np.asarray(
            in1, np.float32
        )
        return gg * (np.float32(imm2) * (gg - 1.0) + np.float32(s0)) + np.asarray(
            in1, np.float32
        ) * np.float32(s0)

    spec = Spec(body=body, reference=ref)
    row = dvo._CUSTOM_DVE_ROW_BASE + len(dvo.OPS)
    shas = {
        ver: DveOpSpec(
            name=_EXP_NAME, opcode=row, uops=lower(spec, ver=ver), rd1_en=True
        ).sha(ver)
        for ver in ("v3", "v4")
    }
    op = dvo.DveOp(_EXP_NAME, spec, subdim=False, uops_sha=shas)
    dvo.OPS.append(op)
    dvo._SUB_OPCODE_FOR_NAME[op.name] = row
    dvo.CUSTOM_DVE_SPECS[op.name] = spec
    return op


EXP_OP = _register_expbits()


def _build():
    nc = bacc.Bacc("TRN2", target_bir_lowering=False, debug=False)
    xt_d = nc.dram_tensor("xt", [B, E, LP], F16, kind="ExternalInput").ap()
    xa_d = nc.dram_tensor("xa", [B, LC, NLC, EA], BF16,
                          kind="ExternalInput").ap()
    lab_d = nc.dram_tensor("labT", [E, NSP], F16, kind="ExternalInput").ap()
    # [EA, B, NSP] so a 2-batch pair is contiguous per partition row
    m_d = nc.dram_tensor("m", [EA, B, NSP], BF16, kind="ExternalOutput").ap()

    with tile.TileContext(nc) as tc, ExitStack() as ctx:
        consts = ctx.enter_context(tc.tile_pool(name="consts", bufs=1))
        xt_pool = ctx.enter_context(tc.tile_pool(name="xtp", bufs=3))
        xa_pool = ctx.enter_context(tc.tile_pool(name="xap", bufs=3))
        e_pool = ctx.enter_context(tc.tile_pool(name="ep", bufs=4))
        j_pool = ctx.enter_context(tc.tile_pool(name="jp", bufs=4))
        et_pool = ctx.enter_context(tc.tile_pool(name="etp", bufs=2))
        u_pool = ctx.enter_context(tc.tile_pool(name="up", bufs=2))
        ps = ctx.enter_context(tc.tile_pool(name="ps", bufs=1, space="PSUM"))

        arena = ps.tile([128, 4096], F32)

        labT = consts.tile([E, NSP], F16)
        nc.sync.dma_start(out=labT[:], in_=lab_d)
        bias_sb = consts.tile([128, 1], F32)
        nc.vector.memset(bias_sb[:], EXP_BIAS)
        warm_sb = consts.tile([128, 512], BF16)
        nc.vector.memset(warm_sb[:], 0.0)
        # ramp the PE's DVFS to full clock while the first inputs stream
        # in: the tensor engine needs ~3us of continuous execution to
        # reach 2.4GHz, and a cold start at 0.65-1.2GHz would drag the
        # first chunks.  bank6 is overwritten by pack_mm1(0) later.
        for _ in range(NWARM):
            nc.tensor.matmul(
                arena[:128, PS0:PS0 + 512], warm_sb[:, 0:128], warm_sb[:])

        xt_tiles, xa_tiles = {}, {}

        def fetch(b):
            xt_tiles[b] = xt_pool.tile([E, LP], F16, tag="xt", name=f"xt{b}")
            xa_tiles[b] = xa_pool.tile(
                [LC, NLC, EA], BF16, tag="xa", name=f"xa{b}")
            if b == 0:
                # batch 0 split into chunked DMAs so mm1(0) can start as
                # soon as the first l-chunk group lands (saves ~4us of head)
                for q in range(4):
                    nc.sync.dma_start(
                        out=xt_tiles[b][:, q * 630:(q + 1) * 630],
                        in_=xt_d[b][:, q * 630:(q + 1) * 630])
                for q in range(2):
                    nc.sync.dma_start(
                        out=xa_tiles[b][:, q * 10:(q + 1) * 10, :],
                        in_=xa_d[b][:, q * 10:(q + 1) * 10, :])
            else:
                nc.sync.dma_start(out=xt_tiles[b][:], in_=xt_d[b])
                nc.sync.dma_start(out=xa_tiles[b][:], in_=xa_d[b])

        fetch(0)
        fetch(1)
        u_pairs = {}

        # ---- emission helpers (close over current-batch tiles) ----------
        def act(b, c):
            """exp of chunk c: ACT covers n[0:NACT), DVE n[NACT:1024)."""
            base = SA0 if c % 2 == 0 else SB0
            e_sb = e_pool.tile([128, NMAIN], BF16, tag="e", name=f"e{b}_{c}")
            nc.scalar.activation(
                e_sb[:LC, :NACT], arena[:LC, base:base + NACT],
                mybir.ActivationFunctionType.Exp,
                bias=bias_sb[:LC], scale=1.0,
            )
            j16 = j_pool.tile([128, W], I16, tag="j", name=f"j{b}_{c}")
            nc.vector.tensor_single_scalar(
                j16[:LC, :], arena[:LC, base + NACT:base + NMAIN],
                B1E, mybir.AluOpType.add,
            )
            nc.vector._custom_dve(
                EXP_OP,
                out=e_sb[:LC, NACT:NMAIN].bitcast(I16),
                in0=arena[:LC, base + NACT:base + NMAIN],
                in1=j16[:LC, :],
                s0=128.0, s1=B1C, imm2=A128,
            )
            return e_sb

        def mm1(xt_tile, c):
            base = SA0 if c % 2 == 0 else SB0
            # j1 FIRST: the VectorE exp chain (E-pass + custom, ~965ns
            # serial -- the longest per-chunk arc) reads only j1's bank
            # [712:1024], so emitting j1 first lets it start ~213ns
            # earlier; ScalarE needs both pieces either way.
            for k, j in enumerate((1, 0)):
                mi = nc.tensor.matmul(
                    arena[:LC, base + j * 512:base + (j + 1) * 512],
                    xt_tile[:, c * LC:(c + 1) * LC],
                    labT[:, j * 512:(j + 1) * 512],
                )
                if k == 1:
                    # same stationary weights as the first piece
                    mi.ins.ldweights = False

        def mm2(xa_sb, e_sb, c):
            for j in range(2):
                mi = nc.tensor.matmul(
                    arena[:EA, U0 + j * 512:U0 + (j + 1) * 512],
                    xa_sb[:, c, :],
                    e_sb[:LC, j * 512:(j + 1) * 512],
                    start=(c == 0), stop=(c == NLC - 1),
                )
                if j == 1:
                    mi.ins.ldweights = False

        def pack_mm1(xt_tile, p):
            for i in range(PACK):
                c = p * PACK + i
                nc.tensor.matmul(
                    arena[:LC, PS0 + i * NT:PS0 + (i + 1) * NT],
                    xt_tile[:, c * LC:(c + 1) * LC],
                    labT[:, NMAIN:NSP],
                )

        def pack_act1(b, p):
            """exp of pack pieces 0-2; pieces 3-4 follow one chunk later so
            the ScalarE load peak is split (a single 460-wide pack ACTIVATE
            on top of the ~700-wide main one blows the per-chunk budget and
            stalls the PE via the score-buffer WAR, resetting its DVFS)."""
            pe = et_pool.tile([128, PACK * NT], BF16, tag="pe",
                              name=f"pe{b}_{p}")
            nc.scalar.activation(
                pe[:LC, :PSPLIT], arena[:LC, PS0:PS0 + PSPLIT],
                mybir.ActivationFunctionType.Exp,
                bias=bias_sb[:LC], scale=1.0,
            )
            return pe

        def pack_act2(pe):
            nc.scalar.activation(
                pe[:LC, PSPLIT:], arena[:LC, PS0 + PSPLIT:PS0 + PACK * NT],
                mybir.ActivationFunctionType.Exp,
                bias=bias_sb[:LC], scale=1.0,
            )

        def pack_mm2(xa_sb, pe, p):
            for i in range(PACK):
                c = p * PACK + i
                nc.tensor.matmul(
                    arena[:EA, UT0:UT0 + NT],
                    xa_sb[:, c, :],
                    pe[:LC, i * NT:(i + 1) * NT],
                    start=(c == 0), stop=(c == NLC - 1),
                )

        # ---- software-pipelined emission over the global chunk stream ---
        # PE slot for chunk c emits [exp(c), mm1(c+1), mm2(c-2)]: exp(c)
        # gets a ~2-slot window before mm2(c), so ACT/DVE throughput jitter
        # and semaphore latency never stall the PE.  mm2 order is just
        # accumulation order, so the extra delay is free.
        mm2_q = []          # pending (xa_sb, e_sb, c_local, b)
        pack_q = []         # pending (xa_sb, pe, p)
        tail_q = []         # deferred last-pack closures
        act2_q = []         # pack-act second halves, fired one chunk later
        pe_last = {}        # b -> pe tile of last pack

        def drain_mm2():
            xa_sb, e_sb, c, b = mm2_q.pop(0)
            mm2(xa_sb, e_sb, c)
            if c == NLC - 1:
                # batch b's main U complete: copy out (bf16) in halves, one
                # per exp engine, so the boundary burst is split and the
                # next batch's mm2(0) [start=True on banks 4,5] only waits
                # per-half; tail cols go with the deferred pack
                u = u_pairs[b // 2]
                nc.scalar.activation(
                    u[:, b % 2, 0:512], arena[:EA, U0:U0 + 512],
                    mybir.ActivationFunctionType.Copy,
                )
                nc.vector.tensor_copy(
                    u[:, b % 2, 512:NMAIN], arena[:EA, U0 + 512:U0 + NMAIN])

        for b in range(B):
            xT = xt_tiles.pop(b)
            xa_sb = xa_tiles.pop(b)
            if b + 2 < B:
                # two batches of prefetch margin: an input DMA's completion
                # can be delayed ~20us when its descriptors land behind an
                # output burst on the shared ring engines
                fetch(b + 2)
            if b % 2 == 0:
                u_pairs[b // 2] = u_pool.tile(
                    [EA, 2, NSP], BF16, tag="u", name=f"u{b//2}")
            u_sb = u_pairs[b // 2]

            if b == 0:
                mm1(xT, 0)

            for c in range(NLC):
                if act2_q:
                    pack_act2(act2_q.pop(0))
                e_sb = act(b, c)
                # PE slot order: mm2(c-2) FIRST (its deps are 2 slots old,
                # always ready), mm1(c+1) second -- the score-buffer WAR
                # wait on exp(c-1) is then absorbed by ~600ns of mm2 work
                # instead of stalling the slot head and exposing the
                # ~170ns SBUF-access latency on every following matmul
                if len(mm2_q) >= 2:
                    drain_mm2()
                if c + 1 < NLC:
                    mm1(xT, c + 1)
                elif b + 1 < B:
                    # hoist next batch's first mm1 ahead of the boundary
                    mm1(xt_tiles[b + 1], 0)
                mm2_q.append((xa_sb, e_sb, c, b))

                if c == 2 and tail_q:
                    # previous batch's last pack + tail copy + output DMA
                    # (after its mm2(19) drained in the c==1 slot)
                    tail_q.pop(0)()

                if c % PACK == PACK - 1:
                    p = c // PACK
                    pack_mm1(xT, p)
                    pe = pack_act1(b, p)
                    act2_q.append(pe)
                    if p == NPACK - 1:
                        pe_last[b] = pe
                    else:
                        pack_q.append((xa_sb, pe, p))
                while pack_q and c >= PACK * pack_q[0][2] + 6:
                    qxa, qpe, qp = pack_q.pop(0)
                    pack_mm2(qxa, qpe, qp)

            # deferred into next batch: last pack mm2, U tail copy, and the
            # output DMA
            def tail(b=b, u=u_sb, xa_cur=xa_sb, pe=pe_last.pop(b)):
                pack_mm2(xa_cur, pe, NPACK - 1)
                nc.vector.tensor_copy(
                    u[:, b % 2, NMAIN:NSP], arena[:EA, UT0:UT0 + NT])
                if b % 2 == 1 and b < 7:
                    # pair complete: partition-split across the two HW DGE
                    # queues (each queue's SBUF->DRAM descriptors serialize
                    # on one engine at ~196ns/packet), and into 26-row
                    # pieces so the write bursts interleave with input
                    # descriptors on the shared ring engines
                    pr = b // 2
                    for lo, hi in ((0, 26), (26, 51)):
                        nc.sync.dma_start(
                            out=m_d[lo:hi, 2 * pr:2 * pr + 2, :],
                            in_=u[lo:hi])
                    for lo, hi in ((51, 76), (76, EA)):
                        nc.scalar.dma_start(
                            out=m_d[lo:hi, 2 * pr:2 * pr + 2, :],
                            in_=u[lo:hi])
                elif b == 6:
                    # ship batch 6 alone so it drains during batch 7
                    nc.sync.dma_start(
                        out=m_d[0:51, 6:7, :], in_=u[0:51, 0:1, :])
                    nc.scalar.dma_start(
                        out=m_d[51:EA, 6:7, :], in_=u[51:EA, 0:1, :])
                elif b == 7:
                    # end-of-run drain: 3-way split incl. the gpsimd sw DGE
                    nc.sync.dma_start(
                        out=m_d[0:34, 7:8, :], in_=u[0:34, 1:2, :])
                    nc.scalar.dma_start(
                        out=m_d[34:68, 7:8, :], in_=u[34:68, 1:2, :])
                    nc.gpsimd.dma_start(
                        out=m_d[68:EA, 7:8, :], in_=u[68:EA, 1:2, :])

            tail_q.append(tail)

        # drain: mm2(18), mm2(19) of batch 7 (emits its U copy), batch 7's
        # last pack-act half, then the deferred tails (last pack, tail
        # copy, final DMAs)
        while act2_q:
            pack_act2(act2_q.pop(0))
        while mm2_q:
            drain_mm2()
        while tail_q:
            tail_q.pop(0)()
    nc.compile()
    return nc


def _get_nc():
    if not _NC:
        _NC.append(_build())
    return _NC[0]


def kernel(x, label_feature):
    global LAST_RESULT
    x = np.ascontiguousarray(np.asarray(x, dtype=np.float32))
    lf = np.ascontiguousarray(np.asarray(label_feature, dtype=np.float32))
    assert x.shape == (B, L, E) and lf.shape == (N_TOTAL, E)

    xa_f = np.zeros((B, LP, EA), np.float32)
    xa_f[:, :L, :E] = x
    xa_f[:, :L, E] = 1.0
    # [B, LP, EA] -> [B, LC, NLC, EA] so the device DMA is contiguous
    xa = np.ascontiguousarray(
        xa_f.reshape(B, NLC, LC, EA).transpose(0, 2, 1, 3)
    ).astype(ml_dtypes.bfloat16)
    xt = np.zeros((B, E, LP), np.float16)
    xt[:, :, :L] = x.transpose(0, 2, 1).astype(np.float16)

    in_maps = []
    for r in range(NCORES):
        lo = r * NS
        hi = min(lo + NS, N_TOTAL)
        labT_f = np.zeros((E, NSP), np.float32)
        labT_f[:, : hi - lo] = lf[lo:hi].T
        # VectorE bit-trick columns get scores pre-scaled by log2(e)
        labT_f[:, NACT:NMAIN] *= LOG2E
        labT = labT_f.astype(np.float16)
        in_maps.append({"xt": xt, "xa": xa, "labT": labT})

    nc = _get_nc()
    res = run_bass_kernel_spmd(
        nc, in_maps, core_ids=list(range(NCORES)), trace=TRACE
    )
    LAST_RESULT = res

    out = np.empty((B, N_TOTAL, E), np.float32)
    for r in range(NCORES):
        lo = r * NS
        hi = min(lo + NS, N_TOTAL)
        u = np.asarray(res.results[r]["m"]).astype(np.float32)  # [EA, B, NSP]
        m = u[:E, :, : hi - lo] / u[E, :, : hi - lo]
        out[:, lo:hi, :] = m.transpose(1, 2, 0)
    return out


# revision 35
# speedup vs baseline: 1.0014x; 1.0014x over previous
"""CodeWiseAttention kernel for Trainium2 (8 NeuronCores, label-dim sharded).

m[b,n,:] = softmax(label_feature[n] @ x[b].T) @ x[b]

Sharding: label rows N=8922 split across 8 cores (1116/core); x replicated.
Per core, per batch b:
  mm1 (fp16):  S^T[l,n] = xT[e,l].T @ labT[e,n]    (xT, labT pre-transposed
               on host; fp16 runs the PE at 1 cycle/row vs ~3.5 for fp32)
  exp:  split across BOTH ScalarE and VectorE to break the ACT-engine
        ridge (ACT alone would need 1116 cols x 0.83ns = 930ns/chunk plus
        ~350ns fixed -- over the ~980ns PE chunk time):
        - ScalarE: exp(s-30) -> bf16 for n[0:712] and the n[1024:1116]
          tail (pack-batched, 5 l-chunks per ACTIVATE).
        - VectorE: n[712:1024] via an exact-exponent bit-trick.  labT for
          these columns is pre-scaled by log2(e) on host so PSUM holds
          s' = s*log2e.  Pass 1 (tensor_single_scalar): E = int16(s'+B-.5)
          (HW converter rounds-to-nearest; the -0.5 pre-bias makes it a
          floor).  Pass 2 (custom 7-stage DVE op ANT_EXPBITS_QC):
          bits = (E + q(g))*128 with g = s'+B - E in [0,1) and
          q(g) = g*(1 + a*(g-1)) ~ 2^g - 1; the int16 bits ARE
          bf16(2^(s'+B-127)).  Per-column scale e^shift cancels in the
          host-side division by Z.  Max |exp err| ~0.6%; end-to-end rel
          err vs f64 softmax measured 6.7e-3 offline (= the bf16-output
          floor; plain Schraudolph at 3% ripple measured 1.4e-2).
  mm2 (bf16):  Uaug^T[e',n] += xa[l,e'].T @ expS^T[l,n] accumulated over l,
               where xa has a ones column so row 100 of Uaug = Z.
  out: VectorE copies Uaug^T -> SBUF as bf16 (paired 2 batches per tile);
       DMA to DRAM m[EA, B, NSP] so each partition's pair is one contiguous
       4464B packet.  All SBUF->DRAM descriptors of one HW DGE queue
       execute on a single DMA engine (~196ns/packet), so pair DMAs are
       split by partitions across the qSP (nc.sync) and qAct (nc.scalar)
       queues; the final batch is 3-way split (sync/scalar/gpsimd-swdge)
       to shorten the end-of-run drain.  Host divides by Z and transposes.

Pipeline: the exp engines run at ~1.0us/chunk, i.e. AT the PE chunk time,
so mm2 for chunk c is emitted TWO chunks late (PE slot c+1 = [mm1(c+2),
mm2(c)]) giving exp(c) a ~2-slot window; semaphore/queue latency then
never stalls the PE.  mm2 order within a batch is just accumulation
order, so the extra delay is free.  The U copy (bank WAR for the next
batch) is split in halves interleaved between the next chunk's two
VectorE exp instructions.

PSUM layout (8 banks x 512 fp32 per partition):
  b0 [0:512]      scores buf A, n[0:512]     } ACT reads [0:712),
  b1 [512:1024]   scores buf A, n[512:1024]  } DVE reads [712:1024)
  b2 [1024:1536]  scores buf B, n[0:512]
  b3 [1536:2048]  scores buf B, n[512:1024]
  b4 [2048:2560]  U accum, n[0:512]
  b5 [2560:3072]  U accum, n[512:1024]
  b6 [3072:3532]  "pack" scores: n[1024:1116] for 5 l-chunks at once
  b7 [3584:3676]  U accum, n[1024:1116]
No bank is PE-written while ScalarE/VectorE reads it (fatal collision);
matmul outputs each sit inside one bank; accumulator banks never see
start=True from score writes.
"""
import numpy as np
import ml_dtypes
from contextlib import ExitStack

import concourse.tile as tile
from concourse import bacc, mybir
from concourse.bass_utils import run_bass_kernel_spmd

F32 = mybir.dt.float32
F16 = mybir.dt.float16
BF16 = mybir.dt.bfloat16
I16 = mybir.dt.int16

B, L, E = 8, 2500, 100
LP = 2520          # L padded; pad rows have xa=0 (incl. ones col) so they
                   # contribute nothing to U or Z even though exp != 0
N_TOTAL = 8922
NCORES = 8
NS = 1116          # label rows per core (core 7: 1110 real)
NSP = 1116
NMAIN = 1024       # n columns in the main (512,512) loop
NT = 92            # tail n columns handled by packs
W = 312            # n columns handled by the VectorE exp bit-trick
NACT = NMAIN - W   # 712: n columns on ScalarE in the main loop
PSPLIT = 276       # pack-act split point (3 pieces / 2 pieces)
NWARM = 18         # PE warm-up matmuls (ramp the DVFS during input DMA)
LC = 126           # l-chunk rows (partition dim of S^T)
NLC = LP // LC     # 20 l-chunks
PACK = 5           # l-chunks per tail pack
NPACK = NLC // PACK
EA = E + 1         # x augmented with ones column
EXP_BIAS = -30.0

LOG2E = 1.4426950408889634
DVE_SHIFT = 28.0                      # e^{s-shift} scale; cancels in Z-div
B1 = 127.0 - DVE_SHIFT * LOG2E        # 86.6045...
B1E = B1 - 0.5                        # pass-1 scalar: rne(u-0.5) == floor(u)
B1C = B1                              # pass-2 C1
A128 = 28.0                           # 128*a of q(g)=g(1+a(g-1))

SA0, SB0 = 0, 1024     # main score buffer offsets (f32 elems)
U0 = 2048              # main U accumulator offset
PS0 = 3072             # pack score offset
UT0 = 3584             # tail U accumulator offset

TRACE = False
LAST_RESULT = None

_NC = []

_EXP_NAME = "ANT_EXPBITS_QC"


def _register_expbits():
    """Register the custom DVE op (documented extension point: append to
    dve_ops.OPS).  bits = g*(C2*(g-1) + C0) + Src1*C0 with g = Src0+C1-Src1;
    7 ALU stages.  Idempotent across calls/processes."""
    import concourse.dve_ops as dvo
    from concourse.dve_spec import Spec, Src0, Src1, C0, C1, C2, One, lower
    from concourse.dve_uop import DveOpSpec

    for op in dvo.OPS:
        if op.name == _EXP_NAME:
            return op

    u = Src0 + C1
    g = u - Src1
    m3 = (g - One) * C2 + C0
    body = g * m3 + Src1 * C0

    def ref(in0, in1, s0, s1, imm2):
        gg = (np.asarray(in0, np.float32) + np.float32(s1)) - np.asarray(
            in1, np.float32
        )
        return gg * (np.float32(imm2) * (gg - 1.0) + np.float32(s0)) + np.asarray(
            in1, np.float32
        ) * np.float32(s0)

    spec = Spec(body=body, reference=ref)
    row = dvo._CUSTOM_DVE_ROW_BASE + len(dvo.OPS)
    shas = {
        ver: DveOpSpec(
            name=_EXP_NAME, opcode=row, uops=lower(spec, ver=ver), rd1_en=True
        ).sha(ver)
        for ver in ("v3", "v4")
    }
    op = dvo.DveOp(_EXP_NAME, spec, subdim=False, uops_sha=shas)
    dvo.OPS.append(op)
    dvo._SUB_OPCODE_FOR_NAME[op.name] = row
    dvo.CUSTOM_DVE_SPECS[op.name] = spec
    return op


EXP_OP = _register_expbits()


def _build():
    nc = bacc.Bacc("TRN2", target_bir_lowering=False, debug=False)
    xt_d = nc.dram_tensor("xt", [B, E, LP], F16, kind="ExternalInput").ap()
    xa_d = nc.dram_tensor("xa", [B, LC, NLC, EA], BF16,
                          kind="ExternalInput").ap()
    lab_d = nc.dram_tensor("labT", [E, NSP], F16, kind="ExternalInput").ap()
    # [EA, B, NSP] so a 2-batch pair is contiguous per partition row
    m_d = nc.dram_tensor("m", [EA, B, NSP], BF16, kind="ExternalOutput").ap()

    with tile.TileContext(nc) as tc, ExitStack() as ctx:
        consts = ctx.enter_context(tc.tile_pool(name="consts", bufs=1))
        xt_pool = ctx.enter_context(tc.tile_pool(name="xtp", bufs=3))
        xa_pool = ctx.enter_context(tc.tile_pool(name="xap", bufs=3))
        e_pool = ctx.enter_context(tc.tile_pool(name="ep", bufs=4))
        j_pool = ctx.enter_context(tc.tile_pool(name="jp", bufs=4))
        et_pool = ctx.enter_context(tc.tile_pool(name="etp", bufs=2))
        u_pool = ctx.enter_context(tc.tile_pool(name="up", bufs=2))
        ps = ctx.enter_context(tc.tile_pool(name="ps", bufs=1, space="PSUM"))

        arena = ps.tile([128, 4096], F32)

        labT = consts.tile([E, NSP], F16)
        nc.sync.dma_start(out=labT[:], in_=lab_d)
        bias_sb = consts.tile([128, 1], F32)
        nc.vector.memset(bias_sb[:], EXP_BIAS)
        warm_sb = consts.tile([128, 512], BF16)
        nc.vector.memset(warm_sb[:], 0.0)
        # ramp the PE's DVFS to full clock while the first inputs stream
        # in: the tensor engine needs ~3us of continuous execution to
        # reach 2.4GHz, and a cold start at 0.65-1.2GHz would drag the
        # first chunks.  bank6 is overwritten by pack_mm1(0) later.
        for _ in range(NWARM):
            nc.tensor.matmul(
                arena[:128, PS0:PS0 + 512], warm_sb[:, 0:128], warm_sb[:])

        xt_tiles, xa_tiles = {}, {}

        def fetch(b):
            xt_tiles[b] = xt_pool.tile([E, LP], F16, tag="xt", name=f"xt{b}")
            xa_tiles[b] = xa_pool.tile(
                [LC, NLC, EA], BF16, tag="xa", name=f"xa{b}")
            if b == 0:
                # batch 0 split into chunked DMAs so mm1(0) can start as
                # soon as the first l-chunk group lands (saves ~4us of head)
                for q in range(4):
                    nc.sync.dma_start(
                        out=xt_tiles[b][:, q * 630:(q + 1) * 630],
                        in_=xt_d[b][:, q * 630:(q + 1) * 630])
                for q in range(2):
                    nc.sync.dma_start(
                        out=xa_tiles[b][:, q * 10:(q + 1) * 10, :],
                        in_=xa_d[b][:, q * 10:(q + 1) * 10, :])
            else:
                nc.sync.dma_start(out=xt_tiles[b][:], in_=xt_d[b])
                nc.sync.dma_start(out=xa_tiles[b][:], in_=xa_d[b])

        fetch(0)
        fetch(1)
        u_pairs = {}

        # ---- emission helpers (close over current-batch tiles) ----------
        def act(b, c):
            """exp of chunk c: ACT covers n[0:NACT), DVE n[NACT:1024)."""
            base = SA0 if c % 2 == 0 else SB0
            e_sb = e_pool.tile([128, NMAIN], BF16, tag="e", name=f"e{b}_{c}")
            nc.scalar.activation(
                e_sb[:LC, :NACT], arena[:LC, base:base + NACT],
                mybir.ActivationFunctionType.Exp,
                bias=bias_sb[:LC], scale=1.0,
            )
            j16 = j_pool.tile([128, W], I16, tag="j", name=f"j{b}_{c}")
            nc.vector.tensor_single_scalar(
                j16[:LC, :], arena[:LC, base + NACT:base + NMAIN],
                B1E, mybir.AluOpType.add,
            )
            nc.vector._custom_dve(
                EXP_OP,
                out=e_sb[:LC, NACT:NMAIN].bitcast(I16),
                in0=arena[:LC, base + NACT:base + NMAIN],
                in1=j16[:LC, :],
                s0=128.0, s1=B1C, imm2=A128,
            )
            return e_sb

        def mm1(xt_tile, c):
            base = SA0 if c % 2 == 0 else SB0
            for j in range(2):
                mi = nc.tensor.matmul(
                    arena[:LC, base + j * 512:base + (j + 1) * 512],
                    xt_tile[:, c * LC:(c + 1) * LC],
                    labT[:, j * 512:(j + 1) * 512],
                )
                if j == 1:
                    # same stationary weights as j==0: skip the reload
                    mi.ins.ldweights = False

        def mm2(xa_sb, e_sb, c):
            for j in range(2):
                mi = nc.tensor.matmul(
                    arena[:EA, U0 + j * 512:U0 + (j + 1) * 512],
                    xa_sb[:, c, :],
                    e_sb[:LC, j * 512:(j + 1) * 512],
                    start=(c == 0), stop=(c == NLC - 1),
                )
                if j == 1:
                    mi.ins.ldweights = False

        def pack_mm1(xt_tile, p):
            for i in range(PACK):
                c = p * PACK + i
                nc.tensor.matmul(
                    arena[:LC, PS0 + i * NT:PS0 + (i + 1) * NT],
                    xt_tile[:, c * LC:(c + 1) * LC],
                    labT[:, NMAIN:NSP],
                )

        def pack_act1(b, p):
            """exp of pack pieces 0-2; pieces 3-4 follow one chunk later so
            the ScalarE load peak is split (a single 460-wide pack ACTIVATE
            on top of the ~700-wide main one blows the per-chunk budget and
            stalls the PE via the score-buffer WAR, resetting its DVFS)."""
            pe = et_pool.tile([128, PACK * NT], BF16, tag="pe",
                              name=f"pe{b}_{p}")
            nc.scalar.activation(
                pe[:LC, :PSPLIT], arena[:LC, PS0:PS0 + PSPLIT],
                mybir.ActivationFunctionType.Exp,
                bias=bias_sb[:LC], scale=1.0,
            )
            return pe

        def pack_act2(pe):
            nc.scalar.activation(
                pe[:LC, PSPLIT:], arena[:LC, PS0 + PSPLIT:PS0 + PACK * NT],
                mybir.ActivationFunctionType.Exp,
                bias=bias_sb[:LC], scale=1.0,
            )

        def pack_mm2(xa_sb, pe, p):
            for i in range(PACK):
                c = p * PACK + i
                nc.tensor.matmul(
                    arena[:EA, UT0:UT0 + NT],
                    xa_sb[:, c, :],
                    pe[:LC, i * NT:(i + 1) * NT],
                    start=(c == 0), stop=(c == NLC - 1),
                )

        # ---- software-pipelined emission over the global chunk stream ---
        # PE slot for chunk c emits [exp(c), mm1(c+1), mm2(c-2)]: exp(c)
        # gets a ~2-slot window before mm2(c), so ACT/DVE throughput jitter
        # and semaphore latency never stall the PE.  mm2 order is just
        # accumulation order, so the extra delay is free.
        mm2_q = []          # pending (xa_sb, e_sb, c_local, b)
        pack_q = []         # pending (xa_sb, pe, p)
        tail_q = []         # deferred last-pack closures (first 3 pieces)
        tail2_q = []        # deferred last-pack closures (pieces 3-4 + out)
        act2_q = []         # pack-act second halves, fired one chunk later
        pe_last = {}        # b -> pe tile of last pack

        def drain_mm2():
            xa_sb, e_sb, c, b = mm2_q.pop(0)
            mm2(xa_sb, e_sb, c)
            if c == NLC - 1:
                # batch b's main U complete: copy out (bf16) in halves, one
                # per exp engine, so the boundary burst is split and the
                # next batch's mm2(0) [start=True on banks 4,5] only waits
                # per-half; tail cols go with the deferred pack
                u = u_pairs[b // 2]
                nc.scalar.activation(
                    u[:, b % 2, 0:512], arena[:EA, U0:U0 + 512],
                    mybir.ActivationFunctionType.Copy,
                )
                nc.vector.tensor_copy(
                    u[:, b % 2, 512:NMAIN], arena[:EA, U0 + 512:U0 + NMAIN])

        for b in range(B):
            xT = xt_tiles.pop(b)
            xa_sb = xa_tiles.pop(b)
            if b + 2 < B:
                # two batches of prefetch margin: an input DMA's completion
                # can be delayed ~20us when its descriptors land behind an
                # output burst on the shared ring engines
                fetch(b + 2)
            if b % 2 == 0:
                u_pairs[b // 2] = u_pool.tile(
                    [EA, 2, NSP], BF16, tag="u", name=f"u{b//2}")
            u_sb = u_pairs[b // 2]

            if b == 0:
                mm1(xT, 0)

            for c in range(NLC):
                if act2_q:
                    pack_act2(act2_q.pop(0))
                e_sb = act(b, c)
                # PE slot order: mm2(c-2) FIRST (its deps are 2 slots old,
                # always ready), mm1(c+1) second -- the score-buffer WAR
                # wait on exp(c-1) is then absorbed by ~600ns of mm2 work
                # instead of stalling the slot head and exposing the
                # ~170ns SBUF-access latency on every following matmul
                if len(mm2_q) >= 2:
                    drain_mm2()
                if c + 1 < NLC:
                    mm1(xT, c + 1)
                elif b + 1 < B:
                    # hoist next batch's first mm1 ahead of the boundary
                    mm1(xt_tiles[b + 1], 0)
                mm2_q.append((xa_sb, e_sb, c, b))

                if c == 2 and tail_q:
                    # previous batch's last pack, split 3+2 across two
                    # slots: a single 5-piece burst (with 5 ldweights)
                    # convoys the PE at the boundary while the exp engines
                    # idle ~1.5us waiting for mm2(19)
                    tail_q.pop(0)()
                elif c == 3 and tail2_q:
                    tail2_q.pop(0)()

                if c % PACK == PACK - 1:
                    p = c // PACK
                    pack_mm1(xT, p)
                    pe = pack_act1(b, p)
                    act2_q.append(pe)
                    if p == NPACK - 1:
                        pe_last[b] = pe
                    else:
                        pack_q.append((xa_sb, pe, p))
                while pack_q and c >= PACK * pack_q[0][2] + 6:
                    qxa, qpe, qp = pack_q.pop(0)
                    pack_mm2(qxa, qpe, qp)

            # deferred into next batch: last pack mm2 (split 3+2 across two
            # slots), U tail copy, and the output DMA
            pe_l = pe_last.pop(b)

            def tail(xa_cur=xa_sb, pe=pe_l):
                p = NPACK - 1
                for i in range(3):
                    c = p * PACK + i
                    nc.tensor.matmul(
                        arena[:EA, UT0:UT0 + NT],
                        xa_cur[:, c, :],
                        pe[:LC, i * NT:(i + 1) * NT],
                        start=(c == 0), stop=(c == NLC - 1),
                    )

            def tail2(b=b, u=u_sb, xa_cur=xa_sb, pe=pe_l):
                p = NPACK - 1
                for i in range(3, PACK):
                    c = p * PACK + i
                    nc.tensor.matmul(
                        arena[:EA, UT0:UT0 + NT],
                        xa_cur[:, c, :],
                        pe[:LC, i * NT:(i + 1) * NT],
                        start=(c == 0), stop=(c == NLC - 1),
                    )
                nc.vector.tensor_copy(
                    u[:, b % 2, NMAIN:NSP], arena[:EA, UT0:UT0 + NT])
                if b % 2 == 1 and b < 7:
                    # pair complete: partition-split across the two HW DGE
                    # queues (each queue's SBUF->DRAM descriptors serialize
                    # on one engine at ~196ns/packet), and into 26-row
                    # pieces so the write bursts interleave with input
                    # descriptors on the shared ring engines
                    pr = b // 2
                    for lo, hi in ((0, 26), (26, 51)):
                        nc.sync.dma_start(
                            out=m_d[lo:hi, 2 * pr:2 * pr + 2, :],
                            in_=u[lo:hi])
                    for lo, hi in ((51, 76), (76, EA)):
                        nc.scalar.dma_start(
                            out=m_d[lo:hi, 2 * pr:2 * pr + 2, :],
                            in_=u[lo:hi])
                elif b == 6:
                    # ship batch 6 alone so it drains during batch 7
                    nc.sync.dma_start(
                        out=m_d[0:51, 6:7, :], in_=u[0:51, 0:1, :])
                    nc.scalar.dma_start(
                        out=m_d[51:EA, 6:7, :], in_=u[51:EA, 0:1, :])
                elif b == 7:
                    # end-of-run drain: 3-way split incl. the gpsimd sw DGE
                    nc.sync.dma_start(
                        out=m_d[0:34, 7:8, :], in_=u[0:34, 1:2, :])
                    nc.scalar.dma_start(
                        out=m_d[34:68, 7:8, :], in_=u[34:68, 1:2, :])
                    nc.gpsimd.dma_start(
                        out=m_d[68:EA, 7:8, :], in_=u[68:EA, 1:2, :])

            tail_q.append(tail)
            tail2_q.append(tail2)

        # drain: mm2(18), mm2(19) of batch 7 (emits its U copy), batch 7's
        # last pack-act half, then the deferred tails (last pack, tail
        # copy, final DMAs)
        while act2_q:
            pack_act2(act2_q.pop(0))
        while mm2_q:
            drain_mm2()
        while tail_q:
            tail_q.pop(0)()
        while tail2_q:
            tail2_q.pop(0)()
    nc.compile()
    return nc


def _get_nc():
    if not _NC:
        _NC.append(_build())
    return _NC[0]


def kernel(x, label_feature):
    global LAST_RESULT
    x = np.ascontiguousarray(np.asarray(x, dtype=np.float32))
    lf = np.ascontiguousarray(np.asarray(label_feature, dtype=np.float32))
    assert x.shape == (B, L, E) and lf.shape == (N_TOTAL, E)

    xa_f = np.zeros((B, LP, EA), np.float32)
    xa_f[:, :L, :E] = x
    xa_f[:, :L, E] = 1.0
    # [B, LP, EA] -> [B, LC, NLC, EA] so the device DMA is contiguous
    xa = np.ascontiguousarray(
        xa_f.reshape(B, NLC, LC, EA).transpose(0, 2, 1, 3)
    ).astype(ml_dtypes.bfloat16)
    xt = np.zeros((B, E, LP), np.float16)
    xt[:, :, :L] = x.transpose(0, 2, 1).astype(np.float16)

    in_maps = []
    for r in range(NCORES):
        lo = r * NS
        hi = min(lo + NS, N_TOTAL)
        labT_f = np.zeros((E, NSP), np.float32)
        labT_f[:, : hi - lo] = lf[lo:hi].T
        # VectorE bit-trick columns get scores pre-scaled by log2(e)
        labT_f[:, NACT:NMAIN] *= LOG2E
        labT = labT_f.astype(np.float16)
        in_maps.append({"xt": xt, "xa": xa, "labT": labT})

    nc = _get_nc()
    res = run_bass_kernel_spmd(
        nc, in_maps, core_ids=list(range(NCORES)), trace=TRACE
    )
    LAST_RESULT = res

    out = np.empty((B, N_TOTAL, E), np.float32)
    for r in range(NCORES):
        lo = r * NS
        hi = min(lo + NS, N_TOTAL)
        u = np.asarray(res.results[r]["m"]).astype(np.float32)  # [EA, B, NSP]
        m = u[:E, :, : hi - lo] / u[E, :, : hi - lo]
        out[:, lo:hi, :] = m.transpose(1, 2, 0)
    return out
